# revision 1
# baseline (speedup 1.0000x reference)
"""MAB (multihead attention block) Trainium2 kernel.

Sharding: 8 cores = 4 batches x 2 query-halves. Each core computes, for its
batch b and query half s (1024 queries), the full 8-head attention block:
    q = Q @ Wq.T + bq ; k = V @ Wk.T + bk ; v = V @ Wv.T   (bv folded out)
    S = q k^T / sqrt(512); masked softmax over keys; O = q + A @ v + bv
    out = O + relu(O @ Wo.T + bo)

On-chip layouts (per core):
  qT, kT  feature-major [512, Nq/Nk]  (heads = 64-row blocks)
  v       token-major   [Nk, 772]     (per-head blocks with a mask column so
                                       the numerator matmul also accumulates
                                       the masked softmax denominator)
  logits  computed as S^T [keys, queries] so exp+mask+denominator work in
          the key-on-partition layout; 2 heads packed into the PE array via
          row tiling (K=64 each).
All matmuls run as float32r (full PE rate at free-dim >= 256).
"""

import math
import os

import numpy as np

import concourse.bass as bass
import concourse.tile as tile
from concourse import bacc, mybir

F32 = mybir.dt.float32
MM_DT = mybir.dt.float32r  # matmul operand dtype (bitcast view of f32)

DIM = 512
NQ = 1024  # queries per core
NK = 2048  # keys per core
H = 8
D = 64
P = 128
FCH = DIM // P  # 4 feature chunks
KD = DIM // P  # 4 contraction chunks
TCH = NK // P  # 16 token/key chunks
QCH = NQ // 512  # 2 query chunks of 512
SCALE = 1.0 / math.sqrt(DIM)

# v_sb per-token-chunk column layout: 4 even-head blocks of 65 (v[64] | mask),
# then 4 odd-head blocks of 128 (mask | zeros[63] | v[64]).
VW = 4 * 65 + 4 * 128  # 772
EVEN_OFF = [65 * i for i in range(4)]
ODD_OFF = [260 + 128 * i for i in range(4)]

INPUT_SPECS = {
    "QT": (DIM, NQ),
    "VT": (DIM, NK),
    "WqT": (DIM, DIM),
    "WkT": (DIM, DIM),
    "WvTp": (DIM, VW),
    "WoT": (DIM, DIM),
    "bq": (DIM,),
    "bk": (DIM,),
    "bv": (DIM,),
    "bo": (DIM,),
    "mask01": (NK,),
    "maskrep": (NK, 4),
}


def _r(ap):
    return ap.bitcast(MM_DT)


def emit(ctx, tc, io):
    """Emit the kernel. io: dict name -> DRAM AP (inputs + 'outT')."""
    nc = tc.nc
    AF = mybir.ActivationFunctionType
    OP = mybir.AluOpType

    consts = ctx.enter_context(tc.tile_pool(name="consts", bufs=1))
    bigs = ctx.enter_context(tc.tile_pool(name="bigs", bufs=1))

    # ---- constants -------------------------------------------------------
    bq_sb = consts.tile([P, FCH], F32)
    nc.sync.dma_start(bq_sb, io["bq"].rearrange("(c p) -> p c", p=P))
    bk_sb = consts.tile([P, FCH], F32)
    nc.sync.dma_start(bk_sb, io["bk"].rearrange("(c p) -> p c", p=P))
    bv_sb = consts.tile([P, FCH], F32)
    nc.sync.dma_start(bv_sb, io["bv"].rearrange("(c p) -> p c", p=P))
    bo_sb = consts.tile([P, FCH], F32)
    nc.sync.dma_start(bo_sb, io["bo"].rearrange("(c p) -> p c", p=P))
    mask_sb = consts.tile([P, TCH], F32)
    nc.sync.dma_start(mask_sb, io["mask01"].rearrange("(c p) -> p c", p=P))
    mrep_sb = consts.tile([P, TCH, 4], F32)
    nc.sync.dma_start(mrep_sb, io["maskrep"].rearrange("(c p) r -> p c r", p=P))

    # warm the ACT exp table early so the ~2.7us table load overlaps DMA
    warm = consts.tile([1, 1], F32)
    nc.vector.memset(warm, 0.0)
    nc.scalar.activation(warm, warm, AF.Exp)

    # ---- weights / inputs ------------------------------------------------
    wvp_sb = consts.tile([P, KD, VW], MM_DT)
    nc.sync.dma_start(wvp_sb, io["WvTp"].bitcast(MM_DT).rearrange("(kd p) f -> p kd f", p=P))
    wq_sb = consts.tile([P, KD, DIM], MM_DT)
    nc.sync.dma_start(wq_sb, io["WqT"].bitcast(MM_DT).rearrange("(kd p) f -> p kd f", p=P))
    wk_sb = consts.tile([P, KD, DIM], MM_DT)
    nc.sync.dma_start(wk_sb, io["WkT"].bitcast(MM_DT).rearrange("(kd p) f -> p kd f", p=P))
    wo_sb = consts.tile([P, KD, DIM], MM_DT)
    nc.sync.dma_start(wo_sb, io["WoT"].bitcast(MM_DT).rearrange("(kd p) f -> p kd f", p=P))

    vt_src = io["VT"].bitcast(MM_DT).rearrange("(kd p) t -> p kd t", p=P)
    qt_src = io["QT"].bitcast(MM_DT).rearrange("(kd p) t -> p kd t", p=P)
    with tc.tile_pool(name="ins", bufs=1) as ins_pool:
        vtin = ins_pool.tile([P, KD, NK], MM_DT)
        for kd in range(KD):
            nc.sync.dma_start(vtin[:, kd, :], vt_src[:, kd, :])
        qtin = ins_pool.tile([P, KD, NQ], MM_DT)
        for kd in range(KD):
            nc.sync.dma_start(qtin[:, kd, :], qt_src[:, kd, :])

        # ---- persistent results ------------------------------------------
        v_sb = bigs.tile([P, TCH, VW], MM_DT)
        qt_sb = bigs.tile([P, FCH, NQ], MM_DT)
        kt_sb = bigs.tile([P, FCH, NK], MM_DT)
        ot_sb = bigs.tile([P, FCH, NQ], MM_DT)

        # ---- v projection (token-major, permuted heads + mask cols) ------
        with tc.tile_pool(name="ps_v", bufs=2, space="PSUM") as ps_v_pool:
            for t in range(TCH):
                ps_v = ps_v_pool.tile([P, VW], F32, tag="psv")
                for kd in range(KD):
                    lhsT = vtin[:, kd, t * P:(t + 1) * P]
                    nc.tensor.matmul(
                        ps_v[:, 0:512], lhsT, wvp_sb[:, kd, 0:512],
                        start=(kd == 0), stop=(kd == KD - 1),
                    )
                    nc.tensor.matmul(
                        ps_v[:, 512:VW], lhsT, wvp_sb[:, kd, 512:VW],
                        start=(kd == 0), stop=(kd == KD - 1),
                    )
                # zero masked tokens (rows); mask cols are 0 here
                nc.vector.tensor_scalar_mul(v_sb[:, t, :], ps_v, mask_sb[:, t:t + 1])
                # write the mask value into the per-head mask columns
                even_cols = v_sb[:, t, 0:260].rearrange("p (e c) -> p e c", c=65)[:, :, 64]
                nc.vector.tensor_copy(even_cols, mrep_sb[:, t, :])
                odd_cols = v_sb[:, t, 260:VW].rearrange("p (o c) -> p o c", c=128)[:, :, 0]
                nc.vector.tensor_copy(odd_cols, mrep_sb[:, t, :])

        # ---- qT / kT projections (feature-major) -------------------------
        with tc.tile_pool(name="ps_p", bufs=4, space="PSUM") as ps_p_pool:
            for fc in range(FCH):
                for qc in range(QCH):
                    ps = ps_p_pool.tile([P, 512], F32, tag="psp")
                    for kd in range(KD):
                        nc.tensor.matmul(
                            ps,
                            wq_sb[:, kd, fc * P:(fc + 1) * P],
                            qtin[:, kd, qc * 512:(qc + 1) * 512],
                            start=(kd == 0), stop=(kd == KD - 1),
                        )
                    nc.vector.tensor_scalar_add(
                        qt_sb[:, fc, qc * 512:(qc + 1) * 512], ps, bq_sb[:, fc:fc + 1]
                    )
            for fc in range(FCH):
                for n in range(NK // 512):
                    ps = ps_p_pool.tile([P, 512], F32, tag="psp")
                    for kd in range(KD):
                        nc.tensor.matmul(
                            ps,
                            wk_sb[:, kd, fc * P:(fc + 1) * P],
                            vtin[:, kd, n * 512:(n + 1) * 512],
                            start=(kd == 0), stop=(kd == KD - 1),
                        )
                    nc.vector.tensor_scalar_add(
                        kt_sb[:, fc, n * 512:(n + 1) * 512], ps, bk_sb[:, fc:fc + 1]
                    )

    # ---- attention -------------------------------------------------------
    att = ctx.enter_context(tc.tile_pool(name="att", bufs=5))
    sm = ctx.enter_context(tc.tile_pool(name="sm", bufs=3))
    dr = ctx.enter_context(tc.tile_pool(name="dr", bufs=2, space="DRAM"))
    from contextlib import ExitStack as _ES

    attps = _ES()
    ps_s_pool = attps.enter_context(tc.tile_pool(name="ps_s", bufs=2, space="PSUM"))
    ps_n_pool = attps.enter_context(tc.tile_pool(name="ps_n", bufs=4, space="PSUM"))

    for pr in range(FCH):  # head pair (2pr, 2pr+1)
        for qc in range(QCH):
            num0 = ps_n_pool.tile([65, 512], F32, tag="num")
            num1 = ps_n_pool.tile([P, 512], F32, tag="num")
            for kc in range(TCH):
                s_ps = ps_s_pool.tile([P, 1024], F32, tag="s")
                for hh in range(2):
                    nc.tensor.matmul(
                        s_ps[:, hh * 512:(hh + 1) * 512],
                        kt_sb[64 * hh:64 * hh + 64, pr, kc * P:(kc + 1) * P],
                        qt_sb[64 * hh:64 * hh + 64, pr, qc * 512:(qc + 1) * 512],
                        start=True, stop=True,
                        tile_position=(64 * hh, 0),
                    )
                es = att.tile([P, 1024], MM_DT, tag="es")
                nc.scalar.activation(es, s_ps, AF.Exp, scale=SCALE)
                nc.tensor.matmul(
                    num0,
                    v_sb[:, kc, EVEN_OFF[pr]:EVEN_OFF[pr] + 65],
                    es[:, 0:512],
                    start=(kc == 0), stop=(kc == TCH - 1),
                )
                nc.tensor.matmul(
                    num1,
                    v_sb[:, kc, ODD_OFF[pr]:ODD_OFF[pr] + 128],
                    es[:, 512:1024],
                    start=(kc == 0), stop=(kc == TCH - 1),
                )
            for hh in range(2):
                num = num0 if hh == 0 else num1
                drow = num[64:65, :] if hh == 0 else num[0:1, :]
                rec = sm.tile([65, 512], F32, tag="rec")
                rslice = rec[64:65, :] if hh == 0 else rec[0:1, :]
                nc.vector.reciprocal(rslice, drow)
                drec = dr.tile([1, 512], F32, tag="drec")
                nc.sync.dma_start(drec, rslice)
                bca = sm.tile([P, 512], F32, tag="bca")
                bsl = slice(64 * hh, 64 * hh + 64)
                nc.sync.dma_start(bca[bsl, :], drec.to_broadcast([64, 512]))
                t1 = sm.tile([P, 512], F32, tag="t1")
                nsl = slice(0, 64) if hh == 0 else slice(64, 128)
                nc.vector.tensor_tensor(t1[bsl, :], num[nsl, :], bca[bsl, :], op=OP.mult)
                nc.vector.scalar_tensor_tensor(
                    ot_sb[bsl, pr, qc * 512:(qc + 1) * 512],
                    t1[bsl, :],
                    bv_sb[bsl, pr:pr + 1],
                    qt_sb[bsl, pr, qc * 512:(qc + 1) * 512].bitcast(F32),
                    op0=OP.add, op1=OP.add,
                )

    attps.close()

    # ---- output projection ----------------------------------------------
    out_dst = io["outT"].rearrange("(fc p) q -> p fc q", p=P)
    with tc.tile_pool(name="ps_u", bufs=2, space="PSUM") as ps_u_pool:
        for ofc in range(FCH):
            for qc in range(QCH):
                ups = ps_u_pool.tile([P, 512], F32, tag="psu")
                for ifc in range(FCH):
                    nc.tensor.matmul(
                        ups,
                        wo_sb[:, ifc, ofc * P:(ofc + 1) * P],
                        ot_sb[:, ifc, qc * 512:(qc + 1) * 512],
                        start=(ifc == 0), stop=(ifc == FCH - 1),
                    )
                r1 = sm.tile([P, 512], F32, tag="r1")
                nc.vector.tensor_scalar(
                    r1, ups, bo_sb[:, ofc:ofc + 1], 0.0, op0=OP.add, op1=OP.max
                )
                fin = sm.tile([P, 512], F32, tag="fin")
                nc.vector.tensor_tensor(
                    fin, r1, ot_sb[:, ofc, qc * 512:(qc + 1) * 512].bitcast(F32), op=OP.add
                )
                nc.sync.dma_start(out_dst[:, ofc, qc * 512:(qc + 1) * 512], fin)


def make_core_inputs(Q, V, mask, Wq, bq, Wk, bk, Wv, bv, Wo, bo, core):
    b, s = divmod(core, 2)
    f32 = np.float32
    QT = np.ascontiguousarray(Q[b, s * NQ:(s + 1) * NQ, :].T, dtype=f32)
    VT = np.ascontiguousarray(V[b].T, dtype=f32)
    WvT = np.ascontiguousarray(Wv.T, dtype=f32)
    WvTp = np.zeros((DIM, VW), dtype=f32)
    for i in range(4):  # even heads 2i
        WvTp[:, EVEN_OFF[i]:EVEN_OFF[i] + 64] = WvT[:, (2 * i) * 64:(2 * i + 1) * 64]
    for i in range(4):  # odd heads 2i+1
        WvTp[:, ODD_OFF[i] + 64:ODD_OFF[i] + 128] = WvT[:, (2 * i + 1) * 64:(2 * i + 2) * 64]
    m01 = mask[b].astype(f32)
    return {
        "QT": QT,
        "VT": VT,
        "WqT": np.ascontiguousarray(Wq.T, dtype=f32),
        "WkT": np.ascontiguousarray(Wk.T, dtype=f32),
        "WvTp": WvTp,
        "WoT": np.ascontiguousarray(Wo.T, dtype=f32),
        "bq": np.asarray(bq, dtype=f32),
        "bk": np.asarray(bk, dtype=f32),
        "bv": np.asarray(bv, dtype=f32),
        "bo": np.asarray(bo, dtype=f32),
        "mask01": m01,
        "maskrep": np.ascontiguousarray(np.repeat(m01[:, None], 4, axis=1)),
    }


_CACHE = {}


def build_program():
    if "nc" in _CACHE:
        return _CACHE["nc"]
    from contextlib import ExitStack

    nc = bacc.Bacc("TRN2", target_bir_lowering=False, debug=False)
    io = {}
    for name, shape in INPUT_SPECS.items():
        io[name] = nc.dram_tensor(name, list(shape), F32, kind="ExternalInput").ap()
    io["outT"] = nc.dram_tensor("outT", [DIM, NQ], F32, kind="ExternalOutput").ap()
    with tile.TileContext(nc) as tc:
        with ExitStack() as ctx:
            emit(ctx, tc, io)
    nc.compile()
    _CACHE["nc"] = nc
    return nc


def kernel(Q, V, mask, Wq, bq, Wk, bk, Wv, bv, Wo, bo):
    from concourse.bass_utils import run_bass_kernel_spmd

    nc = build_program()
    args = (Q, V, mask, Wq, bq, Wk, bk, Wv, bv, Wo, bo)
    in_maps = [make_core_inputs(*args, core=c) for c in range(8)]
    res = run_bass_kernel_spmd(
        nc, in_maps, core_ids=list(range(8)),
        trace=bool(int(os.environ.get("KTRACE", "0"))),
    )
    _CACHE["last_result"] = res
    B = 4
    out = np.empty((B, 2 * NQ, DIM), np.float32)
    for c in range(8):
        b, s = divmod(c, 2)
        out[b, s * NQ:(s + 1) * NQ, :] = res.results[c]["outT"].T
    return out



# revision 5
# speedup vs baseline: 2.2635x; 2.2635x over previous
"""MAB (multihead attention block) Trainium2 kernel, v3.

Sharding: 8 cores = 4 batches x 2 query-halves. Each core computes, for its
batch b and query half s (1024 queries), the full 8-head attention block:
    q = Q @ Wq.T + bq ; k = V @ Wk.T + bk ; v = V @ Wv.T   (bv folded out)
    S = q k^T / sqrt(512); masked softmax over keys; O = q + A @ v + bv
    out = O + relu(O @ Wo.T + bo)

Precision plan (rel-err budget 2e-2):
  - q path stays bf16 (q feeds the output residual directly, so its error
    is not averaged down).
  - V, Wk, Wv inputs and the softmax weights (es) are fp8e4: their
    quantization error is averaged over the 2048-key contraction.
  - fp8 enables DoubleRow matmuls (2 contraction tiles per pass) for the
    v/k projections and the attention numerator.
  - PSUM accumulation is always f32; kt/qt/ot live in bf16.

Schedule: one flat region. The ACT engine streams one exp per key-chunk
step (the 132us floor); all projection work is chopped into <=1us "quarter
passes" and interleaved into the PE slack between the logits/numerator
matmuls of the attention steps. The key mask is folded into the exp bias
column, and the softmax denominator rides along as a ones-column in the v
tile (so the numerator matmul accumulates it for free).
"""

import math
import os

import numpy as np

import concourse.bass as bass
import concourse.tile as tile
from concourse import bacc, mybir

F32 = mybir.dt.float32
BF16 = mybir.dt.bfloat16
FP8 = mybir.dt.float8e4
DR = mybir.MatmulPerfMode.DoubleRow

DIM = 512
NQ = 1024  # queries per core
NK = 2048  # keys per core
P = 128
FCH = DIM // P  # 4 feature chunks (= head pairs)
KD = DIM // P  # 4 contraction chunks (2 DoubleRow pairs)
TCH = NK // P  # 16 token/key chunks
QCH = NQ // 512  # 2 query chunks of 512
SCALE = 1.0 / math.sqrt(DIM)
MASK_NEG = -30000.0

# v_sb per-token-chunk column layout: 4 even-head blocks of 65 (v[64] | one),
# then 4 odd-head blocks of 128 (one | zeros[63] | v[64]), then 12 pad cols
# so the chunk stride is a multiple of 16 bytes (DoubleRow lhsT AP rule).
VW = 4 * 65 + 4 * 128 + 12  # 784
VUSED = 772
EVEN_OFF = [65 * i for i in range(4)]
ODD_OFF = [260 + 128 * i for i in range(4)]

INPUT_SPECS = {
    "QT": ((DIM, NQ), BF16),
    "VT": ((DIM, NK), FP8),
    "WqT": ((DIM, DIM), BF16),
    "WkT": ((DIM, DIM), FP8),
    "WvTp": ((DIM, VUSED), FP8),
    "WoT": ((DIM, DIM), BF16),
    "bq": ((DIM,), F32),
    "bk": ((DIM,), F32),
    "bv": ((DIM,), F32),
    "bo": ((DIM,), F32),
    "mlog": ((NK,), F32),
}


def emit(ctx, tc, io):
    """Emit the kernel. io: dict name -> DRAM AP (inputs + 'outT')."""
    nc = tc.nc
    AF = mybir.ActivationFunctionType
    OP = mybir.AluOpType

    consts = ctx.enter_context(tc.tile_pool(name="consts", bufs=1))
    bigs = ctx.enter_context(tc.tile_pool(name="bigs", bufs=1))

    # ---- small constants -------------------------------------------------
    bq_sb = consts.tile([P, FCH], F32)
    nc.sync.dma_start(bq_sb, io["bq"].rearrange("(c p) -> p c", p=P))
    bk_sb = consts.tile([P, FCH], F32)
    nc.sync.dma_start(bk_sb, io["bk"].rearrange("(c p) -> p c", p=P))
    bv_sb = consts.tile([P, FCH], F32)
    nc.sync.dma_start(bv_sb, io["bv"].rearrange("(c p) -> p c", p=P))
    bo_sb = consts.tile([P, FCH], F32)
    nc.sync.dma_start(bo_sb, io["bo"].rearrange("(c p) -> p c", p=P))
    mlog_sb = consts.tile([P, TCH], F32)
    nc.sync.dma_start(mlog_sb, io["mlog"].rearrange("(c p) -> p c", p=P))

    # warm the ACT exp table early so the ~2.7us table load overlaps DMA
    warm = consts.tile([1, 1], F32)
    nc.vector.memset(warm, 0.0)
    nc.scalar.activation(warm, warm, AF.Exp)

    # ---- weights / inputs ------------------------------------------------
    # DMA order matters: the v projection wants wvp + VT block 0 first.
    wvp_sb = bigs.tile([P, KD, VUSED], FP8)
    nc.sync.dma_start(wvp_sb, io["WvTp"].rearrange("(kd p) f -> p kd f", p=P))

    vt_src = io["VT"].rearrange("(kd p) t -> p kd t", p=P)
    vtin = bigs.tile([P, KD, NK], FP8)
    for blk in range(2):  # blocks 0,1 early (v chunks 0-7 + k quarters 0,1)
        for kd in range(KD):
            sl = slice(blk * 512, (blk + 1) * 512)
            nc.sync.dma_start(vtin[:, kd, sl], vt_src[:, kd, sl])
    wk_sb = bigs.tile([P, KD, DIM], FP8)
    nc.sync.dma_start(wk_sb, io["WkT"].rearrange("(kd p) f -> p kd f", p=P))
    wq_sb = bigs.tile([P, KD, DIM], BF16)
    nc.sync.dma_start(wq_sb, io["WqT"].rearrange("(kd p) f -> p kd f", p=P))
    qt_src = io["QT"].rearrange("(kd p) t -> p kd t", p=P)
    qtin = bigs.tile([P, KD, NQ], BF16)
    for kd in range(KD):
        nc.sync.dma_start(qtin[:, kd, 0:512], qt_src[:, kd, 0:512])
    for blk in range(2, 4):
        for kd in range(KD):
            sl = slice(blk * 512, (blk + 1) * 512)
            nc.sync.dma_start(vtin[:, kd, sl], vt_src[:, kd, sl])
    for kd in range(KD):
        nc.sync.dma_start(qtin[:, kd, 512:1024], qt_src[:, kd, 512:1024])
    wo_sb = bigs.tile([P, KD, DIM], BF16)
    nc.sync.dma_start(wo_sb, io["WoT"].rearrange("(kd p) f -> p kd f", p=P))

    # ---- persistent results ----------------------------------------------
    v_sb = bigs.tile([P, TCH, VW], FP8)
    qt_sb = bigs.tile([P, FCH, NQ], BF16)
    kt_sb = bigs.tile([P, FCH, NK], BF16)
    ot_sb = bigs.tile([P, FCH, NQ], BF16)

    # ---- pools -----------------------------------------------------------
    ps_s = ctx.enter_context(tc.tile_pool(name="ps_s", bufs=2, space="PSUM"))
    ps_n = ctx.enter_context(tc.tile_pool(name="ps_n", bufs=4, space="PSUM"))
    att = ctx.enter_context(tc.tile_pool(name="att", bufs=3))
    sm = ctx.enter_context(tc.tile_pool(name="sm", bufs=2))
    dr = ctx.enter_context(tc.tile_pool(name="dr", bufs=2, space="DRAM"))

    # ---- projection passes (all transient users of the ps_s ring) --------
    def v_pass(t):
        """Project v for key chunk t: token-major [128 tokens, 772]."""
        ps_v = ps_s.tile([P, VUSED], F32, tag="s", padded_shape=[P, 1024],
                         name="ps_v")
        for g in range(2):  # DoubleRow kd pairs
            lhsT = vtin[:, 2 * g:2 * g + 2, t * P:(t + 1) * P]
            nc.tensor.matmul(
                ps_v[:, 0:512], lhsT, wvp_sb[:, 2 * g:2 * g + 2, 0:512],
                start=(g == 0), stop=(g == 1), perf_mode=DR,
            )
            nc.tensor.matmul(
                ps_v[:, 512:VUSED], lhsT, wvp_sb[:, 2 * g:2 * g + 2, 512:VUSED],
                start=(g == 0), stop=(g == 1), perf_mode=DR,
            )
        nc.vector.tensor_copy(v_sb[:, t, 0:VUSED], ps_v)

    def ones_group(g):
        """Set the denominator ones-columns for key chunks 4g..4g+3."""
        ev = v_sb[:, 4 * g:4 * g + 4, 0:260].rearrange(
            "p t (e c) -> p t e c", c=65)[:, :, :, 64]
        nc.vector.memset(ev, 1.0)
        od = v_sb[:, 4 * g:4 * g + 4, 260:772].rearrange(
            "p t (o c) -> p t o c", c=128)[:, :, :, 0]
        nc.vector.memset(od, 1.0)

    def k_quarter(fc, n):
        """Project k features [128fc] for key cols n*512..(n+1)*512 (fp8 DR)."""
        ps = ps_s.tile([P, 512], F32, tag="s", padded_shape=[P, 1024], name="ps_k")
        for g in range(2):
            nc.tensor.matmul(
                ps, wk_sb[:, 2 * g:2 * g + 2, fc * P:(fc + 1) * P],
                vtin[:, 2 * g:2 * g + 2, n * 512:(n + 1) * 512],
                start=(g == 0), stop=(g == 1), perf_mode=DR,
            )
        nc.vector.tensor_scalar_add(
            kt_sb[:, fc, n * 512:(n + 1) * 512], ps, bk_sb[:, fc:fc + 1]
        )

    def q_quarter(fc, n):
        """Project q features [128fc] for query cols n*512.. (bf16)."""
        ps = ps_s.tile([P, 512], F32, tag="s", padded_shape=[P, 1024], name="ps_q")
        for kd in range(KD):
            nc.tensor.matmul(
                ps, wq_sb[:, kd, fc * P:(fc + 1) * P],
                qtin[:, kd, n * 512:(n + 1) * 512],
                start=(kd == 0), stop=(kd == KD - 1),
            )
        nc.vector.tensor_scalar_add(
            qt_sb[:, fc, n * 512:(n + 1) * 512], ps, bq_sb[:, fc:fc + 1]
        )

    out_dst = io["outT"].rearrange("(fc p) q -> p fc q", p=P)

    def out_finish(ups, qc, ofc):
        qsl = slice(qc * 512, (qc + 1) * 512)
        r1 = sm.tile([P, 512], BF16, tag="r1")
        nc.vector.tensor_scalar(
            r1, ups, bo_sb[:, ofc:ofc + 1], 0.0, op0=OP.add, op1=OP.max
        )
        fin = sm.tile([P, 512], F32, tag="fin")
        nc.vector.tensor_tensor(fin, r1, ot_sb[:, ofc, qsl], op=OP.add)
        nc.sync.dma_start(out_dst[:, ofc, qsl], fin)

    def out_quarter(qc, ofc):
        """Full output-projection block for (qc, ofc) via the s ring."""
        qsl = slice(qc * 512, (qc + 1) * 512)
        ups = ps_s.tile([P, 512], F32, tag="s", padded_shape=[P, 1024], name="ups")
        for ifc in range(FCH):
            nc.tensor.matmul(
                ups, wo_sb[:, ifc, ofc * P:(ofc + 1) * P], ot_sb[:, ifc, qsl],
                start=(ifc == 0), stop=(ifc == FCH - 1),
            )
        out_finish(ups, qc, ofc)

    held = {}

    def out_early(qc, ofc):
        """Start out-proj (ifc 0..2) in a held num-ring slot; fc3 not ready yet."""
        qsl = slice(qc * 512, (qc + 1) * 512)
        ups = ps_n.tile([P, 512], F32, tag="num", name=f"ups{ofc}")
        for ifc in range(3):
            nc.tensor.matmul(
                ups, wo_sb[:, ifc, ofc * P:(ofc + 1) * P], ot_sb[:, ifc, qsl],
                start=(ifc == 0), stop=False,
            )
        held[(qc, ofc)] = ups

    def out_late(qc, ofc):
        ups = held.pop((qc, ofc))
        qsl = slice(qc * 512, (qc + 1) * 512)
        nc.tensor.matmul(
            ups, wo_sb[:, 3, ofc * P:(ofc + 1) * P], ot_sb[:, 3, qsl],
            start=False, stop=True,
        )
        out_finish(ups, qc, ofc)

    # ---- attention --------------------------------------------------------
    state = {}

    def att_begin(pr, qc):
        state["num0"] = ps_n.tile([P, 512], F32, tag="num", name="num0")
        state["num1"] = ps_n.tile([P, 512], F32, tag="num", name="num1")

    def att_step(pr, qc, kc):
        s_ps = ps_s.tile([P, 1024], F32, tag="s", name="s_ps")
        for hh in range(2):
            nc.tensor.matmul(
                s_ps[:, hh * 512:(hh + 1) * 512],
                kt_sb[64 * hh:64 * hh + 64, pr, kc * P:(kc + 1) * P],
                qt_sb[64 * hh:64 * hh + 64, pr, qc * 512:(qc + 1) * 512],
                start=True, stop=True,
                tile_position=(64 * hh, 0),
            )
        if kc % 2 == 0:
            state["es2"] = att.tile([P, 2, 1024], FP8, tag="es", name="es2")
        es2 = state["es2"]
        nc.scalar.activation(es2[:, kc % 2, :], s_ps, AF.Exp,
                             bias=mlog_sb[:, kc:kc + 1], scale=SCALE)
        if kc % 2 == 1:
            # fp8 DoubleRow numerator over the (kc-1, kc) chunk pair
            c = kc // 2
            num0, num1 = state["num0"], state["num1"]
            off = EVEN_OFF[pr]
            nc.tensor.matmul(
                num0[0:65, :], v_sb[:, 2 * c:2 * c + 2, off:off + 65],
                es2[:, :, 0:512],
                start=(c == 0), stop=(c == TCH // 2 - 1), perf_mode=DR,
            )
            off = ODD_OFF[pr]
            nc.tensor.matmul(
                num1, v_sb[:, 2 * c:2 * c + 2, off:off + 128],
                es2[:, :, 512:1024],
                start=(c == 0), stop=(c == TCH // 2 - 1), perf_mode=DR,
            )

    def att_tail(pr, qc):
        num0, num1 = state["num0"], state["num1"]
        qsl = slice(qc * 512, (qc + 1) * 512)
        rec0 = sm.tile([65, 512], F32, tag="rec0")
        nc.vector.reciprocal(rec0[64:65, :], num0[64:65, :])
        rec1 = sm.tile([P, 512], F32, tag="rec1")
        nc.vector.reciprocal(rec1[0:1, :], num1[0:1, :])
        dr2 = dr.tile([2, 512], F32, tag="drec")
        nc.sync.dma_start(dr2[0:1, :], rec0[64:65, :])
        nc.sync.dma_start(dr2[1:2, :], rec1[0:1, :])
        bca = sm.tile([P, 512], F32, tag="bca")
        nc.sync.dma_start(bca[0:64, :], dr2[0:1, :].broadcast_to([64, 512]))
        nc.sync.dma_start(bca[64:128, :], dr2[1:2, :].broadcast_to([64, 512]))
        t1 = sm.tile([P, 512], BF16, tag="t1")
        nc.vector.tensor_tensor(t1[0:64, :], num0[0:64, :], bca[0:64, :],
                                op=OP.mult)
        nc.vector.tensor_tensor(t1[64:128, :], num1[64:128, :], bca[64:128, :],
                                op=OP.mult)
        nc.vector.scalar_tensor_tensor(
            ot_sb[:, pr, qsl], t1, bv_sb[:, pr:pr + 1], qt_sb[:, pr, qsl],
            op0=OP.add, op1=OP.add,
        )

    # ---- fused schedule ---------------------------------------------------
    # prologue: v chunks 0-3, k fc0 cols 0-1023, q fc0 qc0
    for t in range(4):
        v_pass(t)
    ones_group(0)
    k_quarter(0, 0)
    k_quarter(0, 1)
    q_quarter(0, 0)

    # per-(qc,pr,kc) extra work interleaved into the attention stream
    inserts = {
        # everything head-pair pr+1 needs must finish inside block pr
        (0, 0, 2): lambda: k_quarter(0, 2),
        (0, 0, 5): lambda: k_quarter(0, 3),
        (0, 0, 10): lambda: k_quarter(1, 0),
        (0, 0, 11): lambda: k_quarter(1, 1),
        (0, 0, 12): lambda: k_quarter(1, 2),
        (0, 0, 13): lambda: k_quarter(1, 3),
        (0, 0, 14): lambda: q_quarter(1, 0),
        (0, 1, 1): lambda: q_quarter(0, 1),
        (0, 1, 3): lambda: q_quarter(1, 1),
        (0, 1, 5): lambda: k_quarter(2, 0),
        (0, 1, 7): lambda: k_quarter(2, 1),
        (0, 1, 9): lambda: k_quarter(2, 2),
        (0, 1, 11): lambda: k_quarter(2, 3),
        (0, 1, 13): lambda: q_quarter(2, 0),
        (0, 2, 2): lambda: q_quarter(2, 1),
        (0, 2, 4): lambda: k_quarter(3, 0),
        (0, 2, 6): lambda: k_quarter(3, 1),
        (0, 2, 8): lambda: k_quarter(3, 2),
        (0, 2, 10): lambda: k_quarter(3, 3),
        (0, 2, 12): lambda: q_quarter(3, 0),
        (0, 3, 2): lambda: q_quarter(3, 1),
        (1, 0, 3): lambda: out_quarter(0, 0),
        (1, 0, 7): lambda: out_quarter(0, 1),
        (1, 0, 11): lambda: out_quarter(0, 2),
        (1, 0, 15): lambda: out_quarter(0, 3),
        (1, 3, 6): lambda: out_early(1, 0),
        (1, 3, 10): lambda: out_early(1, 1),
    }

    for qc in range(QCH):
        for pr in range(FCH):
            att_begin(pr, qc)
            for kc in range(TCH):
                att_step(pr, qc, kc)
                if qc == 0 and pr == 0 and kc < 12:
                    v_pass(kc + 4)
                    if kc % 4 == 3:
                        ones_group(kc // 4 + 1)
                ins = inserts.get((qc, pr, kc))
                if ins is not None:
                    ins()
            att_tail(pr, qc)
    out_late(1, 0)
    out_late(1, 1)
    out_quarter(1, 2)
    out_quarter(1, 3)


def make_core_inputs(Q, V, mask, Wq, bq, Wk, bk, Wv, bv, Wo, bo, core):
    import ml_dtypes

    BF = ml_dtypes.bfloat16
    F8 = ml_dtypes.float8_e4m3fn
    b, s = divmod(core, 2)
    f32 = np.float32
    QT = np.ascontiguousarray(Q[b, s * NQ:(s + 1) * NQ, :].T).astype(BF)
    VT = np.ascontiguousarray(V[b].T).astype(F8)
    WvT = np.ascontiguousarray(Wv.T, dtype=f32)
    WvTp = np.zeros((DIM, VUSED), dtype=f32)
    for i in range(4):  # even heads 2i
        WvTp[:, EVEN_OFF[i]:EVEN_OFF[i] + 64] = WvT[:, (2 * i) * 64:(2 * i + 1) * 64]
    for i in range(4):  # odd heads 2i+1
        WvTp[:, ODD_OFF[i] + 64:ODD_OFF[i] + 128] = WvT[:, (2 * i + 1) * 64:(2 * i + 2) * 64]
    mlog = np.where(np.asarray(mask[b], bool), 0.0, MASK_NEG).astype(f32)
    return {
        "QT": QT,
        "VT": VT,
        "WqT": np.ascontiguousarray(Wq.T).astype(BF),
        "WkT": np.ascontiguousarray(Wk.T).astype(F8),
        "WvTp": WvTp.astype(F8),
        "WoT": np.ascontiguousarray(Wo.T).astype(BF),
        "bq": np.asarray(bq, dtype=f32),
        "bk": np.asarray(bk, dtype=f32),
        "bv": np.asarray(bv, dtype=f32),
        "bo": np.asarray(bo, dtype=f32),
        "mlog": mlog,
    }


_CACHE = {}


def build_program():
    if "nc" in _CACHE:
        return _CACHE["nc"]
    from contextlib import ExitStack

    nc = bacc.Bacc("TRN2", target_bir_lowering=False, debug=False)
    io = {}
    for name, (shape, dt) in INPUT_SPECS.items():
        io[name] = nc.dram_tensor(name, list(shape), dt, kind="ExternalInput").ap()
    io["outT"] = nc.dram_tensor("outT", [DIM, NQ], F32, kind="ExternalOutput").ap()
    with tile.TileContext(nc) as tc:
        with ExitStack() as ctx:
            emit(ctx, tc, io)
    nc.compile()
    _CACHE["nc"] = nc
    return nc


def kernel(Q, V, mask, Wq, bq, Wk, bk, Wv, bv, Wo, bo):
    from concourse.bass_utils import run_bass_kernel_spmd

    nc = build_program()
    args = (Q, V, mask, Wq, bq, Wk, bk, Wv, bv, Wo, bo)
    in_maps = [make_core_inputs(*args, core=c) for c in range(8)]
    res = run_bass_kernel_spmd(
        nc, in_maps, core_ids=list(range(8)),
        trace=bool(int(os.environ.get("KTRACE", "0"))),
    )
    _CACHE["last_result"] = res
    B = 4
    out = np.empty((B, 2 * NQ, DIM), np.float32)
    for c in range(8):
        b, s = divmod(c, 2)
        out[b, s * NQ:(s + 1) * NQ, :] = res.results[c]["outT"].T
    return out
